# revision 9
# baseline (speedup 1.0000x reference)
"""Causal self-attention (B=2, T=2048, D=1024, H=16, RoPE) on 8 trn2 cores.

Sharding: 2-way data parallel over batch x 4-way tensor parallel over heads.
Core c handles batch c//4, heads [4*(c%4), 4*(c%4)+4).

Per-core on-chip pipeline (all matmuls bf16, fp32 PSUM accumulation):
  1. Q^T,K^T = Wqk_slice.T @ x^T   (transposed layout: [head*64, T])
  2. RoPE on Q^T/K^T via 32-partition shifted DVE mults (cos/sin tables
     precomputed on host in transposed layout)
  3. V natural = x @ Wv_slice (reusing x^T tiles as lhsT), with a ones
     column appended per head (rowsum trick)
  4. S^T tile = K^T_chunk.T @ Q^T_chunk, two heads row-tiled concurrently
     (K=64 each at array rows 0-63 / 64-127); causal: skip above-diagonal
  5. exp on ACT with scale=1/Dh**0.5 folded in (no max subtraction: logits
     are ~N(0,1), overflow-impossible); bf16 output; 0/1 mask multiply on
     the 4 diagonal-straddling tile patterns
  6. out'^T[65, q] accumulated over k-chunks: rows 0-63 unnormalized out^T,
     row 64 = softmax denominator (ones column of V')
  7. 1/denominator on DVE, partition-broadcast on GpSimd, normalize+cast
     fused into the PSUM->SBUF move
  8. partial = out_heads^T.T @ Wout_rows -> DRAM [2048, 1024] fp32
Host sums the 4 head-group partials per batch (row-parallel unshard).
"""

import sys

if "/opt/trn_rl_repo" not in sys.path:
    sys.path.insert(0, "/opt/trn_rl_repo")

import numpy as np
import ml_dtypes

BF16 = ml_dtypes.bfloat16

B, T, D, H, Dh = 2, 2048, 1024, 16, 64
HPC = 4          # heads per core
NCORES = 8
ROPE_BASE = 10000.0

QC = 512         # q-chunk (moving free dim)
KC = 128         # k-chunk (S^T partition dim)
NQJ = T // QC    # 4
NKI = T // KC    # 16

_prog_cache = {}


def _build_program():
    from concourse import bacc, tile, mybir

    fp32 = mybir.dt.float32
    bf16 = mybir.dt.bfloat16

    nc = bacc.Bacc(None)

    xT_d = nc.dram_tensor("xT", [D, T], bf16, kind="ExternalInput")
    wqk_d = nc.dram_tensor("wqk", [D, 512], bf16, kind="ExternalInput")
    wv_d = nc.dram_tensor("wv", [D, 256], bf16, kind="ExternalInput")
    wout_d = nc.dram_tensor("wout", [256, D], bf16, kind="ExternalInput")
    cs_d = nc.dram_tensor("cs", [128, T], fp32, kind="ExternalInput")
    sn_d = nc.dram_tensor("sn", [128, T], fp32, kind="ExternalInput")
    mask_d = nc.dram_tensor("mask", [4, 128, QC], bf16, kind="ExternalInput")
    out_d = nc.dram_tensor("out_partial", [T, D], fp32, kind="ExternalOutput")

    with tile.TileContext(nc) as tc:
        with (
            tc.tile_pool(name="const", bufs=1) as cpool,
            tc.tile_pool(name="work", bufs=3) as wpool,
            tc.tile_pool(name="es", bufs=6) as espool,
            tc.tile_pool(name="esm", bufs=4) as esmpool,
            tc.tile_pool(name="small", bufs=4) as spool,
            tc.tile_pool(name="ostage", bufs=3) as opool,
            tc.tile_pool(name="psum", bufs=8, space="PSUM") as pspool,
        ):
            # ---- persistent SBUF tensors ----
            xt = [cpool.tile([128, T], bf16, name=f"xt{i}", tag=f"xt{i}") for i in range(8)]
            wqk = [cpool.tile([128, 512], bf16, name=f"wqk{i}", tag=f"wqk{i}") for i in range(8)]
            wv = [cpool.tile([128, 256], bf16, name=f"wv{i}", tag=f"wv{i}") for i in range(8)]
            wout = [cpool.tile([128, D], bf16, name=f"wout{i}", tag=f"wout{i}") for i in range(2)]
            cs = cpool.tile([128, T], fp32, tag="cs")
            sn = cpool.tile([128, T], fp32, tag="sn")
            masks = [cpool.tile([128, QC], bf16, name=f"mask{i}", tag=f"mask{i}") for i in range(4)]
            # qk_sb: m=0,1 -> Q head-pairs; m=2,3 -> K head-pairs
            qk_sb = [cpool.tile([128, T], bf16, name=f"qk{m}", tag=f"qk{m}") for m in range(4)]
            # v_sb[tj]: per 128-row T-chunk, [V_h | ones] x 4 heads -> 260 cols
            v_sb = [cpool.tile([128, 4 * 65], bf16, name=f"v{tj}", tag=f"v{tj}") for tj in range(NKI)]
            outT_sb = [cpool.tile([128, T], bf16, name=f"ot{hp}", tag=f"ot{hp}") for hp in range(2)]

            # ---- loads ----
            for i in range(8):
                nc.sync.dma_start(xt[i][:], xT_d[i * 128:(i + 1) * 128, :])
                nc.sync.dma_start(wqk[i][:], wqk_d[i * 128:(i + 1) * 128, :])
                nc.sync.dma_start(wv[i][:], wv_d[i * 128:(i + 1) * 128, :])
            for i in range(2):
                nc.sync.dma_start(wout[i][:], wout_d[i * 128:(i + 1) * 128, :])
            nc.sync.dma_start(cs[:], cs_d[:])
            nc.sync.dma_start(sn[:], sn_d[:])
            for i in range(4):
                nc.sync.dma_start(masks[i][:], mask_d[i])

            # DVE observers: touch each const tensor once so the DVE's
            # vector clock records the DMA-queue sems; later PSUM-reading
            # TensorTensor ops then need only their PE wait (the TT ISA
            # struct can't hold a PE wait + a DMA wait when reading PSUM).
            obs = cpool.tile([1, 8], fp32, tag="obs")
            obs_b = cpool.tile([1, 8], bf16, tag="obs_b")
            nc.vector.tensor_copy(obs[:, 0:1], cs[0:1, 0:1])
            nc.vector.tensor_copy(obs[:, 1:2], sn[0:1, 0:1])
            for i in range(4):
                nc.vector.tensor_copy(obs_b[:, i:i + 1], masks[i][0:1, 0:1])
            # ones column for the softmax-denominator broadcast matmul
            ones_sb = cpool.tile([1, 64], fp32, tag="ones_sb")
            nc.vector.memset(ones_sb[:], 1.0)

            # ---- phase 1: Q^T/K^T projection + RoPE ----
            for m in range(4):
                for nj in range(NQJ):
                    ps = pspool.tile([128, QC], fp32, tag="ps")
                    for kc in range(8):
                        nc.tensor.matmul(
                            ps[:],
                            wqk[kc][:, m * 128:(m + 1) * 128],
                            xt[kc][:, nj * QC:(nj + 1) * QC],
                            start=(kc == 0),
                            stop=(kc == 7),
                        )
                    # RoPE: q'[0:32] = q[0:32]*c - q[32:64]*s
                    #       q'[32:64] = q[32:64]*c + q[0:32]*s  (per 64-row head)
                    # sn rows: [-s; s; -s; s]; cs rows: [c; c; c; c]
                    col = slice(nj * QC, (nj + 1) * QC)
                    tmp = wpool.tile([128, QC], fp32, tag="ropetmp")
                    for g in range(2):
                        o = 64 * g
                        nc.vector.tensor_mul(
                            tmp[o:o + 32, :], ps[o + 32:o + 64, :], sn[o:o + 32, col])
                        nc.vector.tensor_mul(
                            tmp[o + 32:o + 64, :], ps[o:o + 32, :], sn[o + 32:o + 64, col])
                    tmp2 = wpool.tile([128, QC], fp32, tag="ropetmp2")
                    nc.vector.tensor_mul(tmp2[:], ps[:], cs[:, col])
                    nc.vector.tensor_add(qk_sb[m][:, col], tmp[:], tmp2[:])

            # ---- phase 1b: V natural + ones columns ----
            for tj in range(NKI):
                vps = pspool.tile([128, 256], fp32, tag="ps")
                for kc in range(8):
                    nc.tensor.matmul(
                        vps[:],
                        xt[kc][:, tj * 128:(tj + 1) * 128],
                        wv[kc][:],
                        start=(kc == 0),
                        stop=(kc == 7),
                    )
                v4 = v_sb[tj][:].rearrange("p (g d) -> p g d", g=4)
                nc.vector.tensor_copy(v4[:, :, 0:64], vps[:].rearrange("p (g d) -> p g d", g=4))
                nc.vector.memset(v4[:, :, 64:65], 1.0)

            # ---- phase 2: attention ----
            for hp in range(2):
                q_t = qk_sb[hp]
                k_t = qk_sb[2 + hp]
                for qj in range(NQJ):
                    nk = 4 * qj + 4
                    out_ps = [pspool.tile([128, QC], fp32, name="outps", tag="ps") for _ in range(2)]
                    for ki in range(nk):
                        for sub in range(2):
                            h = 2 * hp + sub
                            r = slice(64 * sub, 64 * sub + 64)
                            sps = pspool.tile([128, QC], fp32, tag="ps")
                            nc.tensor.matmul(
                                sps[:],
                                k_t[r, ki * 128:(ki + 1) * 128],
                                q_t[r, qj * QC:(qj + 1) * QC],
                                start=True,
                                stop=True,
                            )
                            es = espool.tile([128, QC], bf16, tag="es")
                            nc.scalar.activation(
                                es[:], sps[:],
                                func=mybir.ActivationFunctionType.Exp,
                                scale=0.125,
                            )
                            if ki >= 4 * qj:
                                esm = esmpool.tile([128, QC], bf16, tag="esm")
                                nc.vector.tensor_mul(esm[:], es[:], masks[ki - 4 * qj][:])
                                use = esm
                            else:
                                use = es
                            nc.tensor.matmul(
                                out_ps[sub][0:65, :],
                                v_sb[ki][:, h * 65:h * 65 + 65],
                                use[:],
                                start=(ki == 0),
                                stop=(ki == nk - 1),
                            )
                    for sub in range(2):
                        # 1/denominator -> broadcast to 64 partitions via a
                        # K=1 fp32 matmul (ones[1,64].T @ rec[1,512]) -> SBUF.
                        # Chain keeps every PSUM-reading DVE op at <=1 wait.
                        rec = spool.tile([1, QC], fp32, tag="rec")
                        nc.vector.reciprocal(rec[:], out_ps[sub][64:65, :])
                        bc_ps = pspool.tile([64, QC], fp32, name="bcps", tag="ps")
                        nc.tensor.matmul(bc_ps[:], ones_sb[:], rec[:],
                                         start=True, stop=True)
                        bc = spool.tile([64, QC], fp32, tag="bc")
                        nc.vector.tensor_copy(bc[:], bc_ps[:])
                        nc.vector.tensor_mul(
                            outT_sb[hp][64 * sub:64 * sub + 64, qj * QC:(qj + 1) * QC],
                            out_ps[sub][0:64, :],
                            bc[:],
                        )

            # ---- phase 3: out projection (row-parallel partial) ----
            for tj in range(NKI):
                ost = opool.tile([128, D], fp32, tag="ostage")
                # absorb the slot's DMA-store WAR wait on a cheap SBUF-only
                # memset so the PSUM-reading copies below keep a single wait
                nc.vector.memset(ost[0:1, 0:1], 0.0)
                for nj2 in range(2):
                    ops = pspool.tile([128, QC], fp32, tag="ps")
                    for kc2 in range(2):
                        nc.tensor.matmul(
                            ops[:],
                            outT_sb[kc2][:, tj * 128:(tj + 1) * 128],
                            wout[kc2][:, nj2 * QC:(nj2 + 1) * QC],
                            start=(kc2 == 0),
                            stop=(kc2 == 1),
                        )
                    nc.vector.tensor_copy(ost[:, nj2 * QC:(nj2 + 1) * QC], ops[:])
                nc.sync.dma_start(out_d[tj * 128:(tj + 1) * 128, :], ost[:])

    nc.compile()
    return nc


def _get_program():
    if "nc" not in _prog_cache:
        _prog_cache["nc"] = _build_program()
    return _prog_cache["nc"]


def _host_inputs(x, W_qkv, W_out):
    """Build the 8 per-core input maps."""
    # RoPE tables, transposed layout
    inv_freq = 1.0 / (ROPE_BASE ** (np.arange(0, Dh, 2, dtype=np.float32) / Dh))
    freqs = np.outer(np.arange(T, dtype=np.float32), inv_freq)  # (T, 32)
    c = np.cos(freqs).T.astype(np.float32)  # (32, T)
    s = np.sin(freqs).T.astype(np.float32)
    cs_np = np.concatenate([c, c, c, c], axis=0)          # [128, T]
    sn_np = np.concatenate([-s, s, -s, s], axis=0)        # [128, T]

    # causal 0/1 masks for the 4 diagonal-straddling tile offsets
    kl = np.arange(KC)[:, None]
    ql = np.arange(QC)[None, :]
    mask_np = np.stack(
        [(ql >= kl + o).astype(BF16) for o in (0, 128, 256, 384)], axis=0)

    w4 = W_qkv.reshape(D, 3, H, Dh)
    in_maps = []
    for core in range(NCORES):
        b, hg = core // 4, core % 4
        hs = slice(HPC * hg, HPC * hg + HPC)
        wq = w4[:, 0, hs, :].reshape(D, 256)
        wk = w4[:, 1, hs, :].reshape(D, 256)
        wv = w4[:, 2, hs, :].reshape(D, 256)
        in_maps.append({
            "xT": np.ascontiguousarray(x[b].T).astype(BF16),
            "wqk": np.concatenate([wq, wk], axis=1).astype(BF16),
            "wv": np.ascontiguousarray(wv).astype(BF16),
            "wout": np.ascontiguousarray(W_out[256 * hg:256 * hg + 256, :]).astype(BF16),
            "cs": cs_np,
            "sn": sn_np,
            "mask": mask_np,
        })
    return in_maps


def kernel(x, W_qkv, W_out):
    from concourse.bass_utils import run_bass_kernel_spmd

    x = np.asarray(x)
    W_qkv = np.asarray(W_qkv)
    W_out = np.asarray(W_out)
    nc = _get_program()
    in_maps = _host_inputs(x, W_qkv, W_out)
    res = run_bass_kernel_spmd(nc, in_maps, list(range(NCORES)))
    out = np.zeros((B, T, D), np.float32)
    for core in range(NCORES):
        out[core // 4] += res.results[core]["out_partial"]
    return out
